# revision 39
# baseline (speedup 1.0000x reference)
"""Trainium2 Bass kernel for GQA attention (B=2, L=2048, D=2048, H=16, KV=8, HD=128).

Sharding: tensor-parallel over heads across 8 cores (2 Q heads + 1 KV head per
core), flash-style attention per core, then two AllToAlls (one per local head)
to redistribute from head-sharding to token-sharding before the output
projection (each core computes 512 full output rows; host concatenates).

v2 schedule: phase-separated so the PE runs long same-shape accumulation
chains (216ns/MM streaming) instead of mixed singles (263ns):
  A: QKV projection for both batches (clean 16-MM chains, aux grouped per tile)
  B: attention, hc-major; scores 2 chunks at a time into a 2-bank PSUM tile,
     one batched exp [128,1024] per group (halves ACT instruction overhead),
     single f16 sacc for the softmax denominator
  C: A2A0 issued mid-B, A2A1 issued before the even out-proj half so both
     collectives overlap compute; wo prefetched via host-repacked layout
     (2KB/partition descriptors instead of 256B).
"""
import math
import numpy as np

B, L, D = 2, 2048, 2048
H, KV, HD = 16, 8, 128
NCORES = 8
T = B * L            # 4096 tokens, b-major
TPC = T // NCORES    # 512 tokens per core after A2A
HPC = H // NCORES    # 2 local query heads
EPS = 1e-5
ROPE_BASE = 10000.0
SCALE = HD ** -0.5

TT = 512             # token tile (free dim)
NTT = L // TT        # 4 token tiles per batch
NDC = D // 128       # 16 contraction chunks
NFC = 4              # output col chunks of 128 in qkv proj (2 q heads + k + v)
NG = L // 256        # 8 attention key groups of 2x128 keys

_CACHE = {}


def _rope_tables():
    """cos/sin LUTs [64, L] computed exactly like the jax reference (f32, cpu)."""
    import jax
    import jax.numpy as jnp

    cpu = jax.devices("cpu")[0]
    with jax.default_device(cpu):
        base = ROPE_BASE * 1.0 ** (HD / (HD - 2))
        freqs = base ** (jnp.arange(0, HD, 2, dtype=jnp.float32) / HD)   # [64]
        pos = jnp.arange(L, dtype=jnp.float32)                           # [L]
        angles = pos[:, None] * freqs[None, :]                           # [L, 64]
        cos = np.asarray(jnp.cos(angles), dtype=np.float32).T.copy()     # [64, L]
        sin = np.asarray(jnp.sin(angles), dtype=np.float32).T.copy()
    return cos, sin


def _build_nc():
    import concourse.bass as bass
    import concourse.tile as tile
    import concourse.mybir as mybir
    from concourse import bacc
    from concourse.masks import make_identity
    from contextlib import ExitStack

    f32 = mybir.dt.float32
    f16 = mybir.dt.float16
    Exp = mybir.ActivationFunctionType.Exp
    Ln = mybir.ActivationFunctionType.Ln
    mult = mybir.AluOpType.mult
    add = mybir.AluOpType.add
    sub = mybir.AluOpType.subtract

    from concourse import bacc as _bacc_mod

    if not getattr(_bacc_mod, "_act_table_patch", False):
        _orig_get = _bacc_mod.get_activation_tables

        def _patched_get(arch):
            t = _orig_get(arch)
            exp = mybir.ActivationFunctionType.Exp
            ln = mybir.ActivationFunctionType.Ln
            for name, funcs in t.items():
                if name != "natural_log_exp_and_others":
                    funcs.discard(exp)
                    funcs.discard(ln)
            return t

        _bacc_mod.get_activation_tables = _patched_get
        _bacc_mod._act_table_patch = True

    nc = bacc.Bacc(num_devices=NCORES)

    xT = nc.dram_tensor("xT", [D, T], f16, kind="ExternalInput")
    wqkv = nc.dram_tensor("wqkv", [D, 512], f16, kind="ExternalInput")
    woP = nc.dram_tensor("woP", [NDC, 2, 128, NDC // 2, 128], f16,
                         kind="ExternalInput")
    lcos = nc.dram_tensor("lcos", [64, L], f16, kind="ExternalInput")
    lsin = nc.dram_tensor("lsin", [64, L], f16, kind="ExternalInput")
    qn = nc.dram_tensor("qn", [HD, 1], f32, kind="ExternalInput")
    kn = nc.dram_tensor("kn", [HD, 1], f32, kind="ExternalInput")
    yT = nc.dram_tensor("yT", [D, TPC], f32, kind="ExternalOutput")

    # A2A bounce buffers, one pair per local head chunk
    cc_in = [nc.dram_tensor(f"cc_in{h}", [NCORES, HD, TPC], f16) for h in range(HPC)]
    cc_out = [nc.dram_tensor(f"cc_out{h}", [NCORES, HD, TPC], f16) for h in range(HPC)]

    with tile.TileContext(nc) as tc, ExitStack() as ctx, nc.allow_low_precision(
        reason="f16 tiles; all matmul accumulation is fp32 PSUM"
    ):
        consts = ctx.enter_context(tc.tile_pool(name="consts", bufs=1))
        xtp = ctx.enter_context(tc.tile_pool(name="xtp", bufs=24))
        qkvp = ctx.enter_context(tc.tile_pool(name="qkvp", bufs=1))
        rsp = ctx.enter_context(tc.tile_pool(name="rsp", bufs=6))
        rdp = ctx.enter_context(tc.tile_pool(name="rdp", bufs=4))
        halfp = ctx.enter_context(tc.tile_pool(name="halfp", bufs=8))
        statp = ctx.enter_context(tc.tile_pool(name="statp", bufs=4))
        etp = ctx.enter_context(tc.tile_pool(name="etp", bufs=3))
        saccp = ctx.enter_context(tc.tile_pool(name="saccp", bufs=3))
        attp = ctx.enter_context(tc.tile_pool(name="attp", bufs=4))
        wop = ctx.enter_context(tc.tile_pool(name="wop", bufs=16))
        wop2 = ctx.enter_context(tc.tile_pool(name="wop2", bufs=8))
        yp = ctx.enter_context(tc.tile_pool(name="yp", bufs=2))
        yep = ctx.enter_context(tc.tile_pool(name="yep", bufs=16))

        # PSUM bank budget (8): pacc 2 + G 2x2 + pmisc 1 + poa 1
        pacc = ctx.enter_context(tc.tile_pool(name="pacc", bufs=2, space="PSUM"))
        pG = ctx.enter_context(tc.tile_pool(name="pG", bufs=2, space="PSUM"))
        pmisc = ctx.enter_context(tc.tile_pool(name="pmisc", bufs=1, space="PSUM"))
        poa = ctx.enter_context(tc.tile_pool(name="poa", bufs=1, space="PSUM"))

        # ---- constants ----
        ones_f = consts.tile([128, 128], f32)
        nc.vector.memset(ones_f, 1.0)
        # all-ones stationary: one matmul computes a partition-sum AND
        # broadcasts it to all 128 output partitions
        ones128 = consts.tile([128, 128], f16)
        nc.vector.tensor_copy(out=ones128, in_=ones_f)
        ident = consts.tile([128, 128], f16)
        make_identity(nc, ident)
        eps_t = consts.tile([128, 1], f32)
        nc.vector.memset(eps_t, EPS)

        # weights for qkv projection: per-dc DMAs interleaved with the first
        # tile's x loads (below) so the first chain starts within a few us
        w_sb = consts.tile([128, NDC, 512], f16)
        _wr = wqkv.ap().rearrange("(dc p) f -> p dc f", p=128)
        # sync/scalar for latency-critical streaming loads; gpsimd reserved
        # for transfers gated on collectives (a waiting dma_start blocks its
        # whole engine queue, and scalar paces attention via exp)
        _dmae = [nc.sync, nc.scalar, nc.sync, nc.scalar]

        # LUTs duplicated into both partition halves so rope tensor_tensor
        # ops always see matching base partitions
        cos_sb = consts.tile([128, L], f16)
        nc.gpsimd.dma_start(out=cos_sb[0:64, :], in_=lcos[:, :])
        nc.gpsimd.dma_start(out=cos_sb[64:128, :], in_=lcos[:, :])
        sin_sb = consts.tile([128, L], f16)
        nc.gpsimd.dma_start(out=sin_sb[0:64, :], in_=lsin[:, :])
        nc.gpsimd.dma_start(out=sin_sb[64:128, :], in_=lsin[:, :])
        qn_sb = consts.tile([HD, 1], f32)
        nc.gpsimd.dma_start(out=qn_sb, in_=qn[:, :])
        kn_sb = consts.tile([HD, 1], f32)
        nc.gpsimd.dma_start(out=kn_sb, in_=kn[:, :])

        # ---- persistent activations, per batch ----
        qh_t = [
            [
                qkvp.tile([128, L], f16, tag=f"ag{h}", bufs=2, name=f"qh{h}{b}")
                for b in range(B)
            ]
            for h in range(HPC)
        ]
        kh_t = [
            qkvp.tile([128, L], f16, tag=f"kh{b}", name=f"kh{b}") for b in range(B)
        ]
        v_t = [
            qkvp.tile([128, L // 128, HD], f16, tag=f"v{b}", name=f"v{b}")
            for b in range(B)
        ]

        def proj_tt(b, tt, ptpool, pre=None):
            """QKV projection for one 512-token tile: four clean 16-MM chains
            first (PE streams at fill rate), then rope/rmsnorm/transpose aux.
            `pre`: already-DMA'd leading x chunks (bridges batch boundaries
            where the xtp pool's lookahead runs dry)."""
            pos0 = tt * TT
            tok0 = b * L + tt * TT
            first = (b == 0 and tt == 0)
            xts = []
            for dc in range(NDC):
                if pre is not None and dc < len(pre):
                    xts.append(pre[dc])
                    continue
                xt = xtp.tile([128, TT], f16, tag="xt")
                _dmae[dc % 4].dma_start(
                    out=xt,
                    in_=xT[dc * 128:(dc + 1) * 128, tok0:tok0 + TT],
                )
                if first:
                    # interleave the weight chunk right behind its x chunk so
                    # chain dc can fire as soon as both land
                    _dmae[dc % 4].dma_start(
                        out=w_sb[:, dc, :], in_=_wr[:, dc, :]
                    )
                xts.append(xt)
            rs = []
            for fc in range(NFC):
                pp = pacc.tile([128, TT], f32, tag="pacc", name=f"pp{fc}")
                for dc in range(NDC):
                    nc.tensor.matmul(
                        pp,
                        w_sb[:, dc, fc * 128:(fc + 1) * 128],
                        xts[dc],
                        start=(dc == 0),
                        stop=(dc == NDC - 1),
                    )
                r = rsp.tile([128, TT], f16, tag="rsrc")
                nc.vector.tensor_copy(out=r, in_=pp)
                rs.append(r)
            for fc in range(3):
                rsrc = rs[fc]
                cs_lo = cos_sb[0:64, pos0:pos0 + TT]
                cs_hi = cos_sb[64:128, pos0:pos0 + TT]
                sn_lo = sin_sb[0:64, pos0:pos0 + TT]
                sn_hi = sin_sb[64:128, pos0:pos0 + TT]
                x1 = rsrc[0:64, :]
                x2 = rsrc[64:128, :]
                t1 = halfp.tile([64, TT], f16, tag="half")
                t2 = halfp.tile([64, TT], f16, tag="half")
                t3 = halfp.tile([64, TT], f16, tag="half")
                t4 = halfp.tile([64, TT], f16, tag="half")
                roped = rdp.tile([128, TT], f16, tag="roped")
                nc.vector.tensor_tensor(out=t1, in0=x1, in1=cs_lo, op=mult)
                nc.vector.tensor_tensor(out=t2, in0=x2, in1=sn_hi, op=mult)
                nc.vector.tensor_tensor(
                    out=roped[0:64, :], in0=t1, in1=t2, op=sub
                )
                nc.vector.tensor_tensor(out=t3, in0=x2, in1=cs_hi, op=mult)
                nc.vector.tensor_tensor(out=t4, in0=x1, in1=sn_lo, op=mult)
                nc.vector.tensor_tensor(
                    out=roped[64:128, :], in0=t3, in1=t4, op=add
                )
                # sum of squares over HD via all-ones matmul: the result is
                # already replicated on every output partition (no bcast MM)
                sq = rdp.tile([128, TT], f16, tag="sq")
                nc.vector.tensor_tensor(out=sq, in0=roped, in1=roped, op=mult)
                pss = pmisc.tile([128, TT], f32, tag="pmisc")
                nc.tensor.matmul(pss, ones128, sq, start=True, stop=True)
                # rstd = exp(-0.5*ln(ss/HD + eps)) -- Ln/Exp share one table
                lnt = statp.tile([128, TT], f32, tag="stat")
                nc.scalar.activation(
                    out=lnt, in_=pss, func=Ln, bias=eps_t, scale=1.0 / HD
                )
                srd = statp.tile([128, TT], f16, tag="stat")
                nc.scalar.activation(out=srd, in_=lnt, func=Exp, scale=-0.5)
                # final: out = (roped * norm_w) * rstd
                w_head = qn_sb if fc < 2 else kn_sb
                if fc < 2:
                    dst = qh_t[fc][b][:, pos0:pos0 + TT]
                else:
                    dst = kh_t[b][:, pos0:pos0 + TT]
                nc.vector.scalar_tensor_tensor(
                    out=dst, in0=roped, scalar=w_head, in1=srd,
                    op0=mult, op1=mult,
                )
            # v: transpose to [tok, HD]
            vt = rs[3]
            for i in range(TT // 128):
                pt = ptpool.tile(
                    [128, 128], f16,
                    tag="G" if ptpool is pG else "pmisc", name="pt",
                )
                nc.tensor.transpose(pt, vt[:, i * 128:(i + 1) * 128], ident)
                nc.vector.tensor_copy(out=v_t[b][:, tt * 4 + i, :], in_=pt)

        def att_pair(jobs, popool=None, potag="pacc"):
            """Attention for query tiles. Per 2-chunk key group: 2 QK
            singles into a 2-bank PSUM tile, one batched exp [128,1024],
            2 chained PV matmuls, 2 f16 sacc adds."""
            if popool is None:
                popool = pacc
            st = []
            for hc, b, tqt in jobs:
                qs = qh_t[hc][b][:, tqt * TT:(tqt + 1) * TT]
                po = popool.tile([128, TT], f32, tag=potag, name=f"po{hc}{b}{tqt}")
                sacc = saccp.tile([128, TT], f16, tag="sacc", name=f"sa{hc}{b}{tqt}")
                st.append((hc, b, tqt, qs, po, sacc))
            for g in range(NG):
                ets = []
                for hc, b, tqt, qs, po, sacc in st:
                    G = pG.tile([128, 1024], f32, tag="G")
                    nc.tensor.matmul(
                        G[:, 0:512],
                        kh_t[b][:, (2 * g) * 128:(2 * g + 1) * 128],
                        qs, start=True, stop=True,
                    )
                    nc.tensor.matmul(
                        G[:, 512:1024],
                        kh_t[b][:, (2 * g + 1) * 128:(2 * g + 2) * 128],
                        qs, start=True, stop=True,
                    )
                    et = etp.tile([128, 1024], f16, tag="et")
                    nc.scalar.activation(out=et, in_=G, func=Exp, scale=SCALE)
                    ets.append(et)
                for (hc, b, tqt, qs, po, sacc), et in zip(st, ets):
                    nc.tensor.matmul(
                        po, v_t[b][:, 2 * g, :], et[:, 0:512],
                        start=(g == 0), stop=False,
                    )
                    nc.tensor.matmul(
                        po, v_t[b][:, 2 * g + 1, :], et[:, 512:1024],
                        start=False, stop=(g == NG - 1),
                    )
                    if g == 0:
                        nc.vector.tensor_tensor(
                            out=sacc, in0=et[:, 0:512], in1=et[:, 512:1024], op=add
                        )
                    else:
                        nc.vector.tensor_tensor(
                            out=sacc, in0=sacc, in1=et[:, 0:512], op=add
                        )
                        nc.vector.tensor_tensor(
                            out=sacc, in0=sacc, in1=et[:, 512:1024], op=add
                        )
            for jidx, (hc, b, tqt, qs, po, sacc) in enumerate(st):
                # denominator: all-ones matmul gives the key-sum replicated on
                # every partition; reciprocal; scale PV output directly.
                # In 2-job pairs the second tail borrows the (then idle) poa
                # bank so the two tails don't serialize on one PSUM slot.
                if jidx == 1 and popool is pacc:
                    pd = poa.tile([128, TT], f32, tag="poa", name="pdb")
                else:
                    pd = pmisc.tile([128, TT], f32, tag="pmisc")
                nc.tensor.matmul(pd, ones128, sacc, start=True, stop=True)
                rdf = attp.tile([128, TT], f32, tag="att")
                nc.vector.reciprocal_approx_fast(out=rdf, in_=pd)
                aout = attp.tile([128, TT], f16, tag="att")
                nc.vector.tensor_tensor(out=aout, in0=po, in1=rdf, op=mult)
                j = b * NTT + tqt
                nc.sync.dma_start(out=cc_in[hc][j, :, :], in_=aout)

        wos_e = {}
        wos_o = {}

        def load_wos(dc, parity):
            store = wos_e if parity == 0 else wos_o
            pool = wop if parity == 0 else wop2
            store[dc] = pool.tile(
                [128, NDC // 2, 128], f16, tag=f"wo{parity}", name=f"wos{parity}_{dc}"
            )
            _dmae[dc % 4].dma_start(out=store[dc], in_=woP.ap()[dc, parity])

        # ---- phase A1: proj batch 0 (clean PE chains) ----
        xpre = []
        for tt in range(NTT):
            proj_tt(0, tt, pG)
            if tt == 2:
                # pre-stage batch 1 tile 0's leading x chunks in the (until
                # now unused) yep slots so the batch boundary doesn't starve
                for dc in range(8):
                    xp = yep.tile([128, TT], f16, tag="ye", name=f"xpre{dc}")
                    _dmae[dc % 4].dma_start(
                        out=xp, in_=xT[dc * 128:(dc + 1) * 128, L:L + TT]
                    )
                    xpre.append(xp)

        # ---- phase A2: proj batch 1 overlapped with batch-0 attention ----
        # (single-job attention paced by exp fills the proj phase's spare
        # ACT capacity; its PE work slots into exp-wait gaps)
        for tt in range(NTT):
            proj_tt(1, tt, pmisc, pre=xpre if tt == 0 else None)
            att_pair([(0, 0, tt)], popool=poa, potag="poa")
            if tt == 0:
                for dc in range(NDC):
                    load_wos(dc, 0)

        # ---- phase B2: batch-1 attention, hc-major ----
        att_pair([(0, 1, 0), (0, 1, 1)])
        att_pair([(0, 1, 2), (0, 1, 3)])

        nc.gpsimd.collective_compute(
            "AllToAll",
            mybir.AluOpType.bypass,
            replica_groups=[list(range(NCORES))],
            ins=[cc_in[0].ap()],
            outs=[cc_out[0].ap()],
        )
        ag0a = qkvp.tile([128, 4, TPC], f16, tag="ag0", bufs=2, name="ag0a")
        ag0b = qkvp.tile([128, 4, TPC], f16, tag="ag0", bufs=2, name="ag0b")
        for j in range(8):
            agt = ag0a if j < 4 else ag0b
            nc.gpsimd.dma_start(
                out=agt[:, j % 4, :], in_=cc_out[0].ap()[j, :, :]
            )
        for dc in range(NDC):
            load_wos(dc, 1)

        att_pair([(1, 0, 0), (1, 0, 1)])
        att_pair([(1, 0, 2), (1, 0, 3)])
        att_pair([(1, 1, 0), (1, 1, 1)])
        att_pair([(1, 1, 2), (1, 1, 3)])

        nc.gpsimd.collective_compute(
            "AllToAll",
            mybir.AluOpType.bypass,
            replica_groups=[list(range(NCORES))],
            ins=[cc_in[1].ap()],
            outs=[cc_out[1].ap()],
        )
        ag1a = qkvp.tile([128, 4, TPC], f16, tag="ag1", bufs=2, name="ag1a")
        ag1b = qkvp.tile([128, 4, TPC], f16, tag="ag1", bufs=2, name="ag1b")
        for j in range(8):
            agt = ag1a if j < 4 else ag1b
            # split across gpsimd+sync: both queues are drained by now, and
            # halving the staging latency shortens the post-A2A1 gap
            eng = nc.gpsimd if j % 2 == 0 else nc.sync
            eng.dma_start(
                out=agt[:, j % 4, :], in_=cc_out[1].ap()[j, :, :]
            )

        # ---- phase C: output projection (even half overlaps A2A1) ----
        ye_t = {}
        for dc in range(NDC):
            py = pacc.tile([128, TPC], f32, tag="pacc", name="pye")
            for j in range(NDC // 2):
                srct = ag0a if j < 4 else ag0b
                nc.tensor.matmul(
                    py, wos_e[dc][:, j, :], srct[:, j % 4, :],
                    start=(j == 0), stop=(j == 7),
                )
            ye = yep.tile([128, TPC], f16, tag="ye", name=f"ye{dc}")
            nc.vector.tensor_copy(out=ye, in_=py)
            ye_t[dc] = ye

        for dc in range(NDC):
            py = pacc.tile([128, TPC], f32, tag="pacc", name="pyo")
            for j in range(NDC // 2):
                srct = ag1a if j < 4 else ag1b
                nc.tensor.matmul(
                    py, wos_o[dc][:, j, :], srct[:, j % 4, :],
                    start=(j == 0), stop=(j == 7),
                )
            yt = yp.tile([128, TPC], f32, tag="y")
            nc.vector.tensor_tensor(out=yt, in0=py, in1=ye_t[dc], op=add)
            _dmae[dc % 4].dma_start(out=yT[dc * 128:(dc + 1) * 128, :], in_=yt)

    nc.finalize()
    return nc


def kernel(x, wq, wk, wv, wo, qn_w, kn_w):
    from concourse.bass_utils import run_bass_kernel_spmd

    if "nc" not in _CACHE:
        _CACHE["nc"] = _build_nc()
    nc = _CACHE["nc"]

    x = np.asarray(x, dtype=np.float32)
    wq = np.asarray(wq, dtype=np.float32)
    wk = np.asarray(wk, dtype=np.float32)
    wv = np.asarray(wv, dtype=np.float32)
    wo = np.asarray(wo, dtype=np.float32)
    qn_w = np.asarray(qn_w, dtype=np.float32).reshape(HD, 1).copy()
    kn_w = np.asarray(kn_w, dtype=np.float32).reshape(HD, 1).copy()

    xT = np.ascontiguousarray(x.reshape(T, D).T.astype(np.float16))
    # wo repacked: woP[dc, parity, p, j, m] = wo[(2j+parity)*128 + p, dc*128 + m]
    # so each [128, 8, 128] stationary-set load is 2KB/partition contiguous.
    wo4 = wo.reshape(NDC // 2, 2, 128, NDC, 128)          # [j, par, p, dc, m]
    woP = np.ascontiguousarray(
        wo4.transpose(3, 1, 2, 0, 4).astype(np.float16)   # [dc, par, p, j, m]
    )
    cos, sin = _rope_tables()
    cos = cos.astype(np.float16)
    sin = sin.astype(np.float16)

    in_maps = []
    for c in range(NCORES):
        wqkv_c = np.ascontiguousarray(
            np.concatenate(
                [
                    wq[:, c * HPC * HD:(c + 1) * HPC * HD],
                    wk[:, c * HD:(c + 1) * HD],
                    wv[:, c * HD:(c + 1) * HD],
                ],
                axis=1,
            ).astype(np.float16)
        )
        in_maps.append(
            {
                "xT": xT,
                "wqkv": wqkv_c,
                "woP": woP,
                "lcos": cos,
                "lsin": sin,
                "qn": qn_w,
                "kn": kn_w,
            }
        )

    trace = bool(_CACHE.get("trace"))
    r = run_bass_kernel_spmd(
        nc, in_maps, core_ids=list(range(NCORES)), trace=trace
    )
    _CACHE["last_result"] = r

    y = np.empty((T, D), dtype=np.float32)
    for c in range(NCORES):
        y[c * TPC:(c + 1) * TPC, :] = r.results[c]["yT"].T
    return y.reshape(B, L, D)


# revision 41
# speedup vs baseline: 1.0229x; 1.0229x over previous
"""Trainium2 Bass kernel for GQA attention (B=2, L=2048, D=2048, H=16, KV=8, HD=128).

Sharding: tensor-parallel over heads across 8 cores (2 Q heads + 1 KV head per
core), flash-style attention per core, then two AllToAlls (one per local head)
to redistribute from head-sharding to token-sharding before the output
projection (each core computes 512 full output rows; host concatenates).

v2 schedule: phase-separated so the PE runs long same-shape accumulation
chains (216ns/MM streaming) instead of mixed singles (263ns):
  A: QKV projection for both batches (clean 16-MM chains, aux grouped per tile)
  B: attention, hc-major; scores 2 chunks at a time into a 2-bank PSUM tile,
     one batched exp [128,1024] per group (halves ACT instruction overhead),
     single f16 sacc for the softmax denominator
  C: A2A0 issued mid-B, A2A1 issued before the even out-proj half so both
     collectives overlap compute; wo prefetched via host-repacked layout
     (2KB/partition descriptors instead of 256B).
"""
import math
import numpy as np

B, L, D = 2, 2048, 2048
H, KV, HD = 16, 8, 128
NCORES = 8
T = B * L            # 4096 tokens, b-major
TPC = T // NCORES    # 512 tokens per core after A2A
HPC = H // NCORES    # 2 local query heads
EPS = 1e-5
ROPE_BASE = 10000.0
SCALE = HD ** -0.5

TT = 512             # token tile (free dim)
NTT = L // TT        # 4 token tiles per batch
NDC = D // 128       # 16 contraction chunks
NFC = 4              # output col chunks of 128 in qkv proj (2 q heads + k + v)
NG = L // 256        # 8 attention key groups of 2x128 keys

_CACHE = {}


def _rope_tables():
    """cos/sin LUTs [64, L] computed exactly like the jax reference (f32, cpu)."""
    import jax
    import jax.numpy as jnp

    cpu = jax.devices("cpu")[0]
    with jax.default_device(cpu):
        base = ROPE_BASE * 1.0 ** (HD / (HD - 2))
        freqs = base ** (jnp.arange(0, HD, 2, dtype=jnp.float32) / HD)   # [64]
        pos = jnp.arange(L, dtype=jnp.float32)                           # [L]
        angles = pos[:, None] * freqs[None, :]                           # [L, 64]
        cos = np.asarray(jnp.cos(angles), dtype=np.float32).T.copy()     # [64, L]
        sin = np.asarray(jnp.sin(angles), dtype=np.float32).T.copy()
    return cos, sin


def _build_nc():
    import concourse.bass as bass
    import concourse.tile as tile
    import concourse.mybir as mybir
    from concourse import bacc
    from concourse.masks import make_identity
    from contextlib import ExitStack

    f32 = mybir.dt.float32
    f16 = mybir.dt.float16
    Exp = mybir.ActivationFunctionType.Exp
    Ln = mybir.ActivationFunctionType.Ln
    mult = mybir.AluOpType.mult
    add = mybir.AluOpType.add
    sub = mybir.AluOpType.subtract

    from concourse import bacc as _bacc_mod

    if not getattr(_bacc_mod, "_act_table_patch", False):
        _orig_get = _bacc_mod.get_activation_tables

        def _patched_get(arch):
            t = _orig_get(arch)
            exp = mybir.ActivationFunctionType.Exp
            ln = mybir.ActivationFunctionType.Ln
            for name, funcs in t.items():
                if name != "natural_log_exp_and_others":
                    funcs.discard(exp)
                    funcs.discard(ln)
            return t

        _bacc_mod.get_activation_tables = _patched_get
        _bacc_mod._act_table_patch = True

    nc = bacc.Bacc(num_devices=NCORES)

    xT = nc.dram_tensor("xT", [D, T], f16, kind="ExternalInput")
    wqkv = nc.dram_tensor("wqkv", [D, 512], f16, kind="ExternalInput")
    woP = nc.dram_tensor("woP", [NDC, 2, 128, NDC // 2, 128], f16,
                         kind="ExternalInput")
    lcos = nc.dram_tensor("lcos", [64, L], f16, kind="ExternalInput")
    lsin = nc.dram_tensor("lsin", [64, L], f16, kind="ExternalInput")
    qn = nc.dram_tensor("qn", [HD, 1], f32, kind="ExternalInput")
    kn = nc.dram_tensor("kn", [HD, 1], f32, kind="ExternalInput")
    yT = nc.dram_tensor("yT", [D, TPC], f32, kind="ExternalOutput")

    # A2A bounce buffers, one pair per local head chunk
    cc_in = [nc.dram_tensor(f"cc_in{h}", [NCORES, HD, TPC], f16) for h in range(HPC)]
    cc_out = [nc.dram_tensor(f"cc_out{h}", [NCORES, HD, TPC], f16) for h in range(HPC)]

    with tile.TileContext(nc) as tc, ExitStack() as ctx, nc.allow_low_precision(
        reason="f16 tiles; all matmul accumulation is fp32 PSUM"
    ):
        consts = ctx.enter_context(tc.tile_pool(name="consts", bufs=1))
        xtp = ctx.enter_context(tc.tile_pool(name="xtp", bufs=24))
        qkvp = ctx.enter_context(tc.tile_pool(name="qkvp", bufs=1))
        rsp = ctx.enter_context(tc.tile_pool(name="rsp", bufs=6))
        rdp = ctx.enter_context(tc.tile_pool(name="rdp", bufs=4))
        halfp = ctx.enter_context(tc.tile_pool(name="halfp", bufs=8))
        statp = ctx.enter_context(tc.tile_pool(name="statp", bufs=4))
        etp = ctx.enter_context(tc.tile_pool(name="etp", bufs=3))
        saccp = ctx.enter_context(tc.tile_pool(name="saccp", bufs=3))
        attp = ctx.enter_context(tc.tile_pool(name="attp", bufs=4))
        wop = ctx.enter_context(tc.tile_pool(name="wop", bufs=16))
        wop2 = ctx.enter_context(tc.tile_pool(name="wop2", bufs=8))
        yp = ctx.enter_context(tc.tile_pool(name="yp", bufs=2))
        yep = ctx.enter_context(tc.tile_pool(name="yep", bufs=16))

        # PSUM bank budget (8): pacc 2 + G 2x2 + pmisc 1 + poa 1
        pacc = ctx.enter_context(tc.tile_pool(name="pacc", bufs=2, space="PSUM"))
        pG = ctx.enter_context(tc.tile_pool(name="pG", bufs=2, space="PSUM"))
        pmisc = ctx.enter_context(tc.tile_pool(name="pmisc", bufs=1, space="PSUM"))
        poa = ctx.enter_context(tc.tile_pool(name="poa", bufs=1, space="PSUM"))

        # ---- constants ----
        ones_f = consts.tile([128, 128], f32)
        nc.vector.memset(ones_f, 1.0)
        # all-ones stationary: one matmul computes a partition-sum AND
        # broadcasts it to all 128 output partitions
        ones128 = consts.tile([128, 128], f16)
        nc.vector.tensor_copy(out=ones128, in_=ones_f)
        ident = consts.tile([128, 128], f16)
        make_identity(nc, ident)
        eps_t = consts.tile([128, 1], f32)
        nc.vector.memset(eps_t, EPS)

        # weights for qkv projection: per-dc DMAs interleaved with the first
        # tile's x loads (below) so the first chain starts within a few us
        w_sb = consts.tile([128, NDC, 512], f16)
        _wr = wqkv.ap().rearrange("(dc p) f -> p dc f", p=128)
        # sync/scalar for latency-critical streaming loads; gpsimd reserved
        # for transfers gated on collectives (a waiting dma_start blocks its
        # whole engine queue, and scalar paces attention via exp)
        _dmae = [nc.sync, nc.scalar, nc.sync, nc.scalar]

        # LUTs duplicated into both partition halves so rope tensor_tensor
        # ops always see matching base partitions
        cos_sb = consts.tile([128, L], f16)
        nc.gpsimd.dma_start(out=cos_sb[0:64, :], in_=lcos[:, :])
        nc.gpsimd.dma_start(out=cos_sb[64:128, :], in_=lcos[:, :])
        sin_sb = consts.tile([128, L], f16)
        nc.gpsimd.dma_start(out=sin_sb[0:64, :], in_=lsin[:, :])
        nc.gpsimd.dma_start(out=sin_sb[64:128, :], in_=lsin[:, :])
        qn_sb = consts.tile([HD, 1], f32)
        nc.gpsimd.dma_start(out=qn_sb, in_=qn[:, :])
        kn_sb = consts.tile([HD, 1], f32)
        nc.gpsimd.dma_start(out=kn_sb, in_=kn[:, :])

        # ---- persistent activations, per batch ----
        qh_t = [
            [
                qkvp.tile([128, L], f16, tag=f"ag{h}", bufs=2, name=f"qh{h}{b}")
                for b in range(B)
            ]
            for h in range(HPC)
        ]
        kh_t = [
            qkvp.tile([128, L], f16, tag=f"kh{b}", name=f"kh{b}") for b in range(B)
        ]
        v_t = [
            qkvp.tile([128, L // 128, HD], f16, tag=f"v{b}", name=f"v{b}")
            for b in range(B)
        ]

        def proj_tt(b, tt, ptpool):
            """QKV projection for one 512-token tile: four clean 16-MM chains
            first (PE streams at fill rate), then rope/rmsnorm/transpose aux."""
            pos0 = tt * TT
            tok0 = b * L + tt * TT
            first = (b == 0 and tt == 0)
            xts = []
            for dc in range(NDC):
                xt = xtp.tile([128, TT], f16, tag="xt")
                _dmae[dc % 4].dma_start(
                    out=xt,
                    in_=xT[dc * 128:(dc + 1) * 128, tok0:tok0 + TT],
                )
                if first:
                    # interleave the weight chunk right behind its x chunk so
                    # chain dc can fire as soon as both land
                    _dmae[dc % 4].dma_start(
                        out=w_sb[:, dc, :], in_=_wr[:, dc, :]
                    )
                xts.append(xt)
            rs = []
            for fc in range(NFC):
                pp = pacc.tile([128, TT], f32, tag="pacc", name=f"pp{fc}")
                for dc in range(NDC):
                    nc.tensor.matmul(
                        pp,
                        w_sb[:, dc, fc * 128:(fc + 1) * 128],
                        xts[dc],
                        start=(dc == 0),
                        stop=(dc == NDC - 1),
                    )
                r = rsp.tile([128, TT], f16, tag="rsrc")
                nc.vector.tensor_copy(out=r, in_=pp)
                rs.append(r)
            for fc in range(3):
                rsrc = rs[fc]
                cs_lo = cos_sb[0:64, pos0:pos0 + TT]
                cs_hi = cos_sb[64:128, pos0:pos0 + TT]
                sn_lo = sin_sb[0:64, pos0:pos0 + TT]
                sn_hi = sin_sb[64:128, pos0:pos0 + TT]
                x1 = rsrc[0:64, :]
                x2 = rsrc[64:128, :]
                t1 = halfp.tile([64, TT], f16, tag="half")
                t2 = halfp.tile([64, TT], f16, tag="half")
                t3 = halfp.tile([64, TT], f16, tag="half")
                t4 = halfp.tile([64, TT], f16, tag="half")
                roped = rdp.tile([128, TT], f16, tag="roped")
                nc.vector.tensor_tensor(out=t1, in0=x1, in1=cs_lo, op=mult)
                nc.vector.tensor_tensor(out=t2, in0=x2, in1=sn_hi, op=mult)
                nc.vector.tensor_tensor(
                    out=roped[0:64, :], in0=t1, in1=t2, op=sub
                )
                nc.vector.tensor_tensor(out=t3, in0=x2, in1=cs_hi, op=mult)
                nc.vector.tensor_tensor(out=t4, in0=x1, in1=sn_lo, op=mult)
                nc.vector.tensor_tensor(
                    out=roped[64:128, :], in0=t3, in1=t4, op=add
                )
                # sum of squares over HD via all-ones matmul: the result is
                # already replicated on every output partition (no bcast MM)
                sq = rdp.tile([128, TT], f16, tag="sq")
                nc.vector.tensor_tensor(out=sq, in0=roped, in1=roped, op=mult)
                pss = pmisc.tile([128, TT], f32, tag="pmisc")
                nc.tensor.matmul(pss, ones128, sq, start=True, stop=True)
                # rstd = exp(-0.5*ln(ss/HD + eps)) -- Ln/Exp share one table
                lnt = statp.tile([128, TT], f32, tag="stat")
                nc.scalar.activation(
                    out=lnt, in_=pss, func=Ln, bias=eps_t, scale=1.0 / HD
                )
                srd = statp.tile([128, TT], f16, tag="stat")
                nc.scalar.activation(out=srd, in_=lnt, func=Exp, scale=-0.5)
                # final: out = (roped * norm_w) * rstd
                w_head = qn_sb if fc < 2 else kn_sb
                if fc < 2:
                    dst = qh_t[fc][b][:, pos0:pos0 + TT]
                else:
                    dst = kh_t[b][:, pos0:pos0 + TT]
                nc.vector.scalar_tensor_tensor(
                    out=dst, in0=roped, scalar=w_head, in1=srd,
                    op0=mult, op1=mult,
                )
            # v: transpose to [tok, HD]
            vt = rs[3]
            for i in range(TT // 128):
                pt = ptpool.tile(
                    [128, 128], f16,
                    tag="G" if ptpool is pG else "pmisc", name="pt",
                )
                nc.tensor.transpose(pt, vt[:, i * 128:(i + 1) * 128], ident)
                nc.vector.tensor_copy(out=v_t[b][:, tt * 4 + i, :], in_=pt)

        def att_pair(jobs, popool=None, potag="pacc"):
            """Attention for query tiles. Per 2-chunk key group: 2 QK
            singles into a 2-bank PSUM tile, one batched exp [128,1024],
            2 chained PV matmuls, 2 f16 sacc adds."""
            if popool is None:
                popool = pacc
            st = []
            for hc, b, tqt in jobs:
                qs = qh_t[hc][b][:, tqt * TT:(tqt + 1) * TT]
                po = popool.tile([128, TT], f32, tag=potag, name=f"po{hc}{b}{tqt}")
                sacc = saccp.tile([128, TT], f16, tag="sacc", name=f"sa{hc}{b}{tqt}")
                st.append((hc, b, tqt, qs, po, sacc))
            for g in range(NG):
                ets = []
                for hc, b, tqt, qs, po, sacc in st:
                    G = pG.tile([128, 1024], f32, tag="G")
                    nc.tensor.matmul(
                        G[:, 0:512],
                        kh_t[b][:, (2 * g) * 128:(2 * g + 1) * 128],
                        qs, start=True, stop=True,
                    )
                    nc.tensor.matmul(
                        G[:, 512:1024],
                        kh_t[b][:, (2 * g + 1) * 128:(2 * g + 2) * 128],
                        qs, start=True, stop=True,
                    )
                    et = etp.tile([128, 1024], f16, tag="et")
                    nc.scalar.activation(out=et, in_=G, func=Exp, scale=SCALE)
                    ets.append(et)
                for (hc, b, tqt, qs, po, sacc), et in zip(st, ets):
                    nc.tensor.matmul(
                        po, v_t[b][:, 2 * g, :], et[:, 0:512],
                        start=(g == 0), stop=False,
                    )
                    nc.tensor.matmul(
                        po, v_t[b][:, 2 * g + 1, :], et[:, 512:1024],
                        start=False, stop=(g == NG - 1),
                    )
                    if g == 0:
                        nc.vector.tensor_tensor(
                            out=sacc, in0=et[:, 0:512], in1=et[:, 512:1024], op=add
                        )
                    else:
                        nc.vector.tensor_tensor(
                            out=sacc, in0=sacc, in1=et[:, 0:512], op=add
                        )
                        nc.vector.tensor_tensor(
                            out=sacc, in0=sacc, in1=et[:, 512:1024], op=add
                        )
            for jidx, (hc, b, tqt, qs, po, sacc) in enumerate(st):
                # denominator: all-ones matmul gives the key-sum replicated on
                # every partition; reciprocal; scale PV output directly.
                # In 2-job pairs the second tail borrows the (then idle) poa
                # bank so the two tails don't serialize on one PSUM slot.
                if jidx == 1 and popool is pacc:
                    pd = poa.tile([128, TT], f32, tag="poa", name="pdb")
                else:
                    pd = pmisc.tile([128, TT], f32, tag="pmisc")
                nc.tensor.matmul(pd, ones128, sacc, start=True, stop=True)
                rdf = attp.tile([128, TT], f32, tag="att")
                nc.vector.reciprocal_approx_fast(out=rdf, in_=pd)
                aout = attp.tile([128, TT], f16, tag="att")
                nc.vector.tensor_tensor(out=aout, in0=po, in1=rdf, op=mult)
                j = b * NTT + tqt
                nc.sync.dma_start(out=cc_in[hc][j, :, :], in_=aout)

        wos_e = {}
        wos_o = {}

        def load_wos(dc, parity):
            store = wos_e if parity == 0 else wos_o
            pool = wop if parity == 0 else wop2
            store[dc] = pool.tile(
                [128, NDC // 2, 128], f16, tag=f"wo{parity}", name=f"wos{parity}_{dc}"
            )
            _dmae[dc % 4].dma_start(out=store[dc], in_=woP.ap()[dc, parity])

        # ---- phase A1: proj batch 0 (clean PE chains) ----
        for tt in range(NTT):
            proj_tt(0, tt, pG)

        # ---- phase A2: proj batch 1 overlapped with batch-0 attention ----
        # (single-job attention paced by exp fills the proj phase's spare
        # ACT capacity; its PE work slots into exp-wait gaps)
        for tt in range(NTT):
            proj_tt(1, tt, pmisc)
            att_pair([(0, 0, tt)], popool=poa, potag="poa")
            if tt == 0:
                for dc in range(NDC):
                    load_wos(dc, 0)

        # ---- phase B2: batch-1 attention, hc-major ----
        att_pair([(0, 1, 0), (0, 1, 1)])
        att_pair([(0, 1, 2), (0, 1, 3)])

        nc.gpsimd.collective_compute(
            "AllToAll",
            mybir.AluOpType.bypass,
            replica_groups=[list(range(NCORES))],
            ins=[cc_in[0].ap()],
            outs=[cc_out[0].ap()],
        )
        ag0a = qkvp.tile([128, 4, TPC], f16, tag="ag0", bufs=2, name="ag0a")
        ag0b = qkvp.tile([128, 4, TPC], f16, tag="ag0", bufs=2, name="ag0b")
        for j in range(8):
            agt = ag0a if j < 4 else ag0b
            nc.gpsimd.dma_start(
                out=agt[:, j % 4, :], in_=cc_out[0].ap()[j, :, :]
            )
        for dc in range(NDC):
            load_wos(dc, 1)

        att_pair([(1, 0, 0), (1, 0, 1)])
        att_pair([(1, 0, 2), (1, 0, 3)])
        att_pair([(1, 1, 0), (1, 1, 1)])
        att_pair([(1, 1, 2), (1, 1, 3)])

        nc.gpsimd.collective_compute(
            "AllToAll",
            mybir.AluOpType.bypass,
            replica_groups=[list(range(NCORES))],
            ins=[cc_in[1].ap()],
            outs=[cc_out[1].ap()],
        )
        ag1a = qkvp.tile([128, 4, TPC], f16, tag="ag1", bufs=2, name="ag1a")
        ag1b = qkvp.tile([128, 4, TPC], f16, tag="ag1", bufs=2, name="ag1b")
        for j in range(8):
            agt = ag1a if j < 4 else ag1b
            # split across gpsimd+sync: both queues are drained by now, and
            # halving the staging latency shortens the post-A2A1 gap
            eng = nc.gpsimd if j % 2 == 0 else nc.sync
            eng.dma_start(
                out=agt[:, j % 4, :], in_=cc_out[1].ap()[j, :, :]
            )

        # ---- phase C: output projection (even half overlaps A2A1) ----
        ye_t = {}
        for dc in range(NDC):
            py = pacc.tile([128, TPC], f32, tag="pacc", name="pye")
            for j in range(NDC // 2):
                srct = ag0a if j < 4 else ag0b
                nc.tensor.matmul(
                    py, wos_e[dc][:, j, :], srct[:, j % 4, :],
                    start=(j == 0), stop=(j == 7),
                )
            ye = yep.tile([128, TPC], f16, tag="ye", name=f"ye{dc}")
            nc.vector.tensor_copy(out=ye, in_=py)
            ye_t[dc] = ye

        for dc in range(NDC):
            py = pacc.tile([128, TPC], f32, tag="pacc", name="pyo")
            for j in range(NDC // 2):
                srct = ag1a if j < 4 else ag1b
                nc.tensor.matmul(
                    py, wos_o[dc][:, j, :], srct[:, j % 4, :],
                    start=(j == 0), stop=(j == 7),
                )
            yt = yp.tile([128, TPC], f32, tag="y")
            nc.vector.tensor_tensor(out=yt, in0=py, in1=ye_t[dc], op=add)
            _dmae[dc % 4].dma_start(out=yT[dc * 128:(dc + 1) * 128, :], in_=yt)

    nc.finalize()
    return nc


def kernel(x, wq, wk, wv, wo, qn_w, kn_w):
    from concourse.bass_utils import run_bass_kernel_spmd

    if "nc" not in _CACHE:
        _CACHE["nc"] = _build_nc()
    nc = _CACHE["nc"]

    x = np.asarray(x, dtype=np.float32)
    wq = np.asarray(wq, dtype=np.float32)
    wk = np.asarray(wk, dtype=np.float32)
    wv = np.asarray(wv, dtype=np.float32)
    wo = np.asarray(wo, dtype=np.float32)
    qn_w = np.asarray(qn_w, dtype=np.float32).reshape(HD, 1).copy()
    kn_w = np.asarray(kn_w, dtype=np.float32).reshape(HD, 1).copy()

    xT = np.ascontiguousarray(x.reshape(T, D).T.astype(np.float16))
    # wo repacked: woP[dc, parity, p, j, m] = wo[(2j+parity)*128 + p, dc*128 + m]
    # so each [128, 8, 128] stationary-set load is 2KB/partition contiguous.
    wo4 = wo.reshape(NDC // 2, 2, 128, NDC, 128)          # [j, par, p, dc, m]
    woP = np.ascontiguousarray(
        wo4.transpose(3, 1, 2, 0, 4).astype(np.float16)   # [dc, par, p, j, m]
    )
    cos, sin = _rope_tables()
    cos = cos.astype(np.float16)
    sin = sin.astype(np.float16)

    in_maps = []
    for c in range(NCORES):
        wqkv_c = np.ascontiguousarray(
            np.concatenate(
                [
                    wq[:, c * HPC * HD:(c + 1) * HPC * HD],
                    wk[:, c * HD:(c + 1) * HD],
                    wv[:, c * HD:(c + 1) * HD],
                ],
                axis=1,
            ).astype(np.float16)
        )
        in_maps.append(
            {
                "xT": xT,
                "wqkv": wqkv_c,
                "woP": woP,
                "lcos": cos,
                "lsin": sin,
                "qn": qn_w,
                "kn": kn_w,
            }
        )

    trace = bool(_CACHE.get("trace"))
    r = run_bass_kernel_spmd(
        nc, in_maps, core_ids=list(range(NCORES)), trace=trace
    )
    _CACHE["last_result"] = r

    y = np.empty((T, D), dtype=np.float32)
    for c in range(NCORES):
        y[c * TPC:(c + 1) * TPC, :] = r.results[c]["yT"].T
    return y.reshape(B, L, D)
